# revision 7
# baseline (speedup 1.0000x reference)
"""Multi-head attention Trainium2 kernel (B=4, S=2048, E=512, H=8).

Sharding: 8 cores = 4 batches x 2 head-groups (4 heads each). Each core gets
the full sequence for one batch and computes QKV projection + attention for
its 4 heads. All transposes are done host-side in numpy (inputs are fed
pre-transposed; the output is returned in a transposed layout and fixed up on
host). The softmax denominator rides the attn@v matmul as a ones-column of v,
and the final division + v-bias add happen on host.

Device-side math notes:
  - k-bias drops out of softmax (constant along the key axis), v-bias is a
    post-softmax additive constant (applied on host), only q-bias is applied.
  - exp() is split between ScalarE (true exp) and VectorE (Schraudolph
    bit-trick exp: one multiply-add writing int16, read back as bf16).
  - all matmuls are bf16 so everything streams at 1 column/cycle.
"""

import math
import sys

import numpy as np

for _p in ("/opt/trn_rl_repo",):
    if _p not in sys.path:
        sys.path.insert(0, _p)

import ml_dtypes  # noqa: E402

import concourse.bass as bass  # noqa: E402
import concourse.mybir as mybir  # noqa: E402
import concourse.tile as tile  # noqa: E402
from concourse import bacc  # noqa: E402
from concourse.bass_utils import run_bass_kernel_spmd  # noqa: E402

B, S, E, H = 4, 2048, 512, 8
DH = 64          # head dim
HC = 4           # heads per core
NPAIR = 2        # head pairs per core
F = HC * 3 * DH  # 768 W rows per core
SCALE = 1.0 / math.sqrt(float(E))
LOG2E = 1.4426950408889634

# Schraudolph exp in bf16 bits: exp(s*SCALE) ~= bitcast_bf16(int16(A*s + B))
EXP_A = (1 << 7) * SCALE * LOG2E
EXP_B = 16249.0  # 127*2^7 - 7.3 (zero-mean log error)

F32 = mybir.dt.float32
BF16 = mybir.dt.bfloat16
I16 = mybir.dt.int16

SK_CHUNKS = S // 128     # 16
SQ_BLOCKS = S // 1024    # 2 blocks of 1024 query positions

# fraction of exp tiles that go to ScalarE (rest go to VectorE Schraudolph)
ACT_NUM, ACT_DEN = 4, 8


def build_mha_kernel(nc: bass.Bass, tc: "tile.TileContext", act_frac=(ACT_NUM, ACT_DEN)):
    xt = nc.dram_tensor("xt", [E, S], BF16, kind="ExternalInput").ap()
    wt = nc.dram_tensor("wt", [E, F], BF16, kind="ExternalInput").ap()
    bq = nc.dram_tensor("bq", [128, NPAIR], F32, kind="ExternalInput").ap()
    out = nc.dram_tensor("out", [HC, DH + 1, S], F32, kind="ExternalOutput").ap()

    act_num, act_den = act_frac
    exp_counter = [0]

    def exp_engine():
        i = exp_counter[0]
        exp_counter[0] += 1
        return "act" if (i % act_den) < act_num else "dve"

    pools = []

    def mkpool(**kw):
        p = tc.alloc_tile_pool(**kw)
        pools.append(p)
        return p

    consts = mkpool(name="consts", bufs=1)
    xpool = mkpool(name="xpool", bufs=1)
    qkpool = mkpool(name="qkpool", bufs=1)
    vpool = mkpool(name="vpool", bufs=1)
    apool = mkpool(name="apool", bufs=3)
    opool = mkpool(name="opool", bufs=2)
    psum = mkpool(name="psum", bufs=1, space="PSUM")

    # ---- load inputs -----------------------------------------------------
    xt_sb = []
    for c in range(4):
        t = xpool.tile([128, S], BF16, name=f"xt_sb{c}", tag=f"xt{c}")
        nc.sync.dma_start(out=t, in_=xt[c * 128:(c + 1) * 128, :])
        xt_sb.append(t)
    wt_sb = []
    for c in range(4):
        t = xpool.tile([128, F], BF16, name=f"wt_sb{c}", tag=f"wt{c}")
        nc.sync.dma_start(out=t, in_=wt[c * 128:(c + 1) * 128, :])
        wt_sb.append(t)
    bq_sb = consts.tile([128, NPAIR], F32, name="bq_sb")
    nc.sync.dma_start(out=bq_sb, in_=bq)

    # wt columns are host-permuted: [pair0 q(128) | pair0 k(128) |
    #                                pair1 q(128) | pair1 k(128) | v(256)]
    def wq(c, j):
        return wt_sb[c][:, j * 256:j * 256 + 128]

    def wk(c, j):
        return wt_sb[c][:, j * 256 + 128:j * 256 + 256]

    def wv(c):
        return wt_sb[c][:, 512:768]

    # ---- v projection: natural [s, f] orientation ------------------------
    # vsb[sk]: [128, HC*65] f32; per head 64 v columns + a ones column.
    vsb = []
    for sk in range(SK_CHUNKS):
        v = vpool.tile([128, HC * (DH + 1)], BF16, name=f"vsb{sk}", tag=f"v{sk}")
        nc.vector.memset(v.rearrange("p (h x) -> p h x", x=DH + 1)[:, :, DH:DH + 1], 1.0)
        vsb.append(v)

    for sk in range(SK_CHUNKS):
        vp = psum.tile([128, HC * DH], F32, name=f"vp{sk}", tag="po_e")
        for c in range(4):
            nc.tensor.matmul(
                vp,
                lhsT=xt_sb[c][:, sk * 128:(sk + 1) * 128],
                rhs=wv(c),
                start=(c == 0),
                stop=(c == 3),
            )
        dst = vsb[sk].rearrange("p (h x) -> p h x", x=DH + 1)[:, :, 0:DH]
        if sk % 2 == 0:
            nc.scalar.copy(dst, vp)
        else:
            nc.vector.tensor_copy(dst, vp)

    # ---- q/k projection: transposed [f, s] orientation, head-pair packed -
    # qt_sb[j]: [128, S] bf16, partitions 0-63 = head 2j qT, 64-127 = head 2j+1 qT
    qt_sb, kt_sb = [], []
    for j in range(NPAIR):
        qt = qkpool.tile([128, S], BF16, name=f"qt_sb{j}", tag=f"qt{j}")
        kt = qkpool.tile([128, S], BF16, name=f"kt_sb{j}", tag=f"kt{j}")
        qt_sb.append(qt)
        kt_sb.append(kt)

    for j in range(NPAIR):
        for sc in range(4):  # s chunks of 512
            ssl = bass.ts(sc, 512)
            qp = psum.tile([128, 512], F32, name=f"qp{j}_{sc}", tag="sc_e")
            kp = psum.tile([128, 512], F32, name=f"kp{j}_{sc}", tag="sc_o")
            for c in range(4):
                nc.tensor.matmul(
                    qp,
                    lhsT=wq(c, j),
                    rhs=xt_sb[c][:, ssl],
                    start=(c == 0),
                    stop=(c == 3),
                )
            for c in range(4):
                nc.tensor.matmul(
                    kp,
                    lhsT=wk(c, j),
                    rhs=xt_sb[c][:, ssl],
                    start=(c == 0),
                    stop=(c == 3),
                )
            # q gets bias (per-partition), k needs none (softmax-invariant)
            nc.vector.tensor_scalar(
                out=qt_sb[j][:, ssl], in0=qp,
                scalar1=bq_sb[:, j:j + 1], scalar2=None,
                op0=mybir.AluOpType.add,
            )
            nc.scalar.copy(kt_sb[j][:, ssl], kp)

    # ---- attention -------------------------------------------------------
    for j in range(NPAIR):
        for sqb in range(SQ_BLOCKS):
            sq = bass.ds(sqb * 1024, 1024)
            sq0 = bass.ds(sqb * 1024, 512)
            sq1 = bass.ds(sqb * 1024 + 512, 512)
            po = []  # per head parity: psum [65, 1024] accumulating out.T|denom
            for t in range(2):
                p = psum.tile([DH + 1, 1024], F32, name=f"po{j}_{sqb}_{t}",
                              tag=f"po_{'e' if t == 0 else 'o'}")
                po.append(p)

            attn_tiles = [[None] * SK_CHUNKS for _ in range(2)]

            def emit_scores(sk, j=j, sq0=sq0, sq1=sq1, attn_tiles=attn_tiles):
                for t in range(2):
                    par = "e" if t == 0 else "o"
                    ps = psum.tile([128, 1024], F32, name=f"sc{j}_{sk}_{t}",
                                   tag=f"sc_{par}")
                    lo, hi = 64 * t, 64 * t + 64
                    for half, sqh in ((0, sq0), (1, sq1)):
                        nc.tensor.matmul(
                            ps[:, half * 512:(half + 1) * 512],
                            lhsT=kt_sb[j][lo:hi, sk * 128:(sk + 1) * 128],
                            rhs=qt_sb[j][lo:hi, sqh],
                            start=True, stop=True,
                        )
                    at = apool.tile([128, 1024], BF16, name=f"at{j}_{sk}_{t}",
                                    tag=f"at_{par}")
                    if exp_engine() == "act":
                        nc.scalar.activation(
                            at, ps, mybir.ActivationFunctionType.Exp, scale=SCALE)
                    else:
                        nc.vector.tensor_scalar(
                            out=at.bitcast(I16), in0=ps,
                            scalar1=float(EXP_A), scalar2=float(EXP_B),
                            op0=mybir.AluOpType.mult, op1=mybir.AluOpType.add,
                        )
                    attn_tiles[t][sk] = at

            def emit_av(sk, j=j, po=po, attn_tiles=attn_tiles):
                for t in range(2):
                    h = 2 * j + t
                    at = attn_tiles[t][sk]
                    for half in range(2):
                        nc.tensor.matmul(
                            po[t][:, half * 512:(half + 1) * 512],
                            lhsT=vsb[sk][:, h * (DH + 1):(h + 1) * (DH + 1)],
                            rhs=at[:, half * 512:(half + 1) * 512],
                            start=(sk == 0), stop=(sk == SK_CHUNKS - 1),
                        )

            # software pipeline: scores(sk) ... av(sk-1)
            for sk in range(SK_CHUNKS):
                emit_scores(sk)
                if sk > 0:
                    emit_av(sk - 1)
            emit_av(SK_CHUNKS - 1)

            for t in range(2):
                h = 2 * j + t
                ob = opool.tile([DH + 1, 1024], F32, name=f"ob{j}_{sqb}_{t}",
                                tag=f"ob_{'e' if t == 0 else 'o'}")
                if t == 0:
                    nc.scalar.copy(ob, po[t])
                else:
                    nc.vector.tensor_copy(ob, po[t])
                nc.sync.dma_start(out=out[h, :, sq], in_=ob)

    for p in reversed(pools):
        p.release()
    return nc


_CACHE = {}


def _get_compiled():
    if "nc" not in _CACHE:
        nc = bacc.Bacc("TRN2", target_bir_lowering=False, debug=False,
                       num_devices=8)
        with tile.TileContext(nc) as tc:
            build_mha_kernel(nc, tc)
        nc.compile()
        _CACHE["nc"] = nc
    return _CACHE["nc"]


def _w_perm():
    """Feature permutation: [pair0 q|pair0 k|pair1 q|pair1 k|v] (see wq/wk/wv)."""
    perm = []
    for j in range(NPAIR):
        for t in range(2):
            h = 2 * j + t
            perm += list(range(h * 192, h * 192 + 64))          # q
        for t in range(2):
            h = 2 * j + t
            perm += list(range(h * 192 + 64, h * 192 + 128))    # k
    for h in range(HC):
        perm += list(range(h * 192 + 128, h * 192 + 192))       # v
    return np.array(perm)


_W_PERM = _w_perm()


def make_in_maps(sentences, Wqkv, bqkv):
    in_maps = []
    for core in range(8):
        b, g = core // 2, core % 2
        xt = np.ascontiguousarray(sentences[b].T).astype(ml_dtypes.bfloat16)
        wt = np.ascontiguousarray(Wqkv[g * F:(g + 1) * F][_W_PERM].T).astype(ml_dtypes.bfloat16)
        bq = np.zeros((128, NPAIR), np.float32)
        for j in range(NPAIR):
            for t in range(2):
                h = 2 * j + t
                off = g * F + h * 3 * DH
                bq[t * 64:(t + 1) * 64, j] = bqkv[off:off + DH]
        in_maps.append({"xt": xt, "wt": wt, "bq": bq})
    return in_maps


def assemble_output(results, bqkv):
    """results[core]["out"]: [HC, 65, S] -> full [B, S, E] output."""
    out = np.empty((B, S, E), np.float32)
    for core in range(8):
        b, g = core // 2, core % 2
        r = results[core]["out"]
        o = r[:, :DH, :] / r[:, DH:DH + 1, :]        # [HC, DH, S]
        o = o.transpose(2, 0, 1).reshape(S, HC * DH)  # [S, 256]
        bias_v = np.concatenate(
            [bqkv[g * F + h * 3 * DH + 2 * DH: g * F + h * 3 * DH + 3 * DH]
             for h in range(HC)])
        out[b, :, g * 256:(g + 1) * 256] = o + bias_v[None, :]
    return out


def kernel(sentences, Wqkv, bqkv):
    nc = _get_compiled()
    in_maps = make_in_maps(np.asarray(sentences, np.float32),
                           np.asarray(Wqkv, np.float32),
                           np.asarray(bqkv, np.float32))
    res = run_bass_kernel_spmd(nc, in_maps, core_ids=list(range(8)))
    return assemble_output(res.results, np.asarray(bqkv, np.float32))
